# revision 21
# baseline (speedup 1.0000x reference)
"""Trainium2 Bass kernel for the rumor-GCN masked-autoencoder loss (v4).

Strategy (8 NeuronCores, SPMD single NEFF):
  - Layer 1 message passing is HOST-PREGATHERED: for each core/direction the
    host materializes xe[slot] = x[src[slot]] in (dst-block, k-tile) slot
    order, shipped pre-tiled [128, KT*512] bf16, streamed with large
    sequential DMAs -- zero gather descriptors for layer 1.
  - All one-hot S matrices (GCN norms dinv[src]*dinv[dst] folded into the
    values; "on"-encoder masking folded by zeroing masked-src rows) are
    PRE-BUILT ON THE HOST and streamed as bf16 -- no on-chip generation.
  - L1 aggregation in normal form: A[node, 512] += S_v^T @ xe per k-tile
    (two wide matmuls); A transposed chunkwise on the PE; W1/W2 applied per
    4-block group with wide matmuls in feature-major form; z2 transposed
    back per block for the AllGather and stashed in SBUF.
  - Layer 2: z2 AllGathered (bf16); dma_gather per (8-block supergroup, 32K
    window) with (4-block group, window) k-tile buckets; aggregation with
    host-shipped narrow per-(tile, block) band S tiles into per-block
    PSUM accumulators (node-major, no transposes); self-loop terms from the
    SBUF z2 stash via host-shipped diag tiles (no gather slots).
  - Pooling via host-shipped batch-onehot matmuls into persistent PSUM; the
    masked-node cosine loss is computed per block inside the L2-BU loop
    (wide-tile tail), one small AllReduce finishes the scalar loss.
"""

import sys

import numpy as np

sys.path.insert(0, "/opt/trn_rl_repo")

# ---------------------------------------------------------------- config


class Cfg:
    def __init__(self, N, E, G, M, C=8, WIN=28672, GB1=4, SGF=2):
        self.N, self.E, self.G, self.M, self.C = N, E, G, M, C
        self.IN, self.HID, self.OUT = 512, 128, 64
        self.WIN = WIN
        self.GB1 = GB1          # blocks per psum group
        self.SGF = SGF          # L2: psum groups per gather supergroup
        assert N % C == 0
        self.OWN = N // C
        self.NB = -(-self.OWN // 128)
        self.OWNP = self.NB * 128
        self.NPAD = C * self.OWNP
        self.NW2 = -(-self.NPAD // WIN)
        self.NG = -(-self.NB // GB1)
        self.NSG = -(-self.NG // SGF)


FULL = Cfg(N=200000, E=400000, G=128, M=100000)

_WNAMES = [p + s for p in ("on_td", "on_bu", "tgt_td", "tgt_bu")
           for s in ("_W1", "_b1", "_W2", "_b2")]


def _rep16(idx_flat):
    n = len(idx_flat)
    assert n % 16 == 0
    blk = np.zeros((16, n // 16), dtype=np.int16)
    k = np.arange(n)
    blk[k % 16, k // 16] = idx_flat
    return np.tile(blk, (8, 1))


def _bcast(vec, parts=128):
    return np.broadcast_to(np.asarray(vec, np.float32)[None, :],
                           (parts, len(vec))).copy()


def _tile_rows(a, ncols):
    kt = a.shape[0] // 128
    return np.ascontiguousarray(
        a.reshape(kt, 128, ncols).transpose(1, 0, 2).reshape(128, kt * ncols))


# ---------------------------------------------------------------- host prep


def _assign_nodes(cfg, wtd, wbu):
    """Node -> (core, block, lane): snake-deal to cores, then capped greedy
    packing into blocks so per-(block, dir) slot loads stay <= CAP (=3
    k-tiles) for nearly every block."""
    import heapq
    c = cfg
    CAP = 384
    w = wtd + wbu
    order = np.argsort(-w, kind="stable")
    core = np.empty(c.N, np.int64)
    grid = np.arange(c.N) % (2 * c.C)
    snake = np.where(grid < c.C, grid, 2 * c.C - 1 - grid)
    core[order] = snake
    blk = np.empty(c.N, np.int64)
    lane = np.empty(c.N, np.int64)
    std = wtd + 1
    sbu = wbu + 1
    for ci in range(c.C):
        nodes = order[core[order] == ci]      # weight-desc
        ltd = [0] * c.NB
        lbu = [0] * c.NB
        cnt = [0] * c.NB
        heap = [(0, b) for b in range(c.NB)]
        heapq.heapify(heap)
        for v in nodes:
            a, b_ = int(std[v]), int(sbu[v])
            popped = []
            pick = None
            while heap:
                key, b = heapq.heappop(heap)
                if key != ltd[b] + lbu[b]:
                    continue                  # stale entry
                popped.append(b)
                if (cnt[b] < 128 and ltd[b] + a <= CAP
                        and lbu[b] + b_ <= CAP):
                    pick = b
                    break
                if len(popped) >= 12:
                    break
            if pick is None:                  # spill: least-loaded with room
                for b in popped:
                    if cnt[b] < 128:
                        pick = b
                        break
            if pick is None:
                pick = min(range(c.NB), key=lambda x: (cnt[x] >= 128,
                                                       ltd[x] + lbu[x]))
            blk[v] = pick
            lane[v] = cnt[pick]
            cnt[pick] += 1
            ltd[pick] += a
            lbu[pick] += b_
            for b in popped:
                if cnt[b] < 128:
                    heapq.heappush(heap, (ltd[b] + lbu[b], b))
    assert lane.max() < 128
    return core, blk, lane


def host_prep(cfg, inp):
    import ml_dtypes
    bf16 = ml_dtypes.bfloat16
    c = cfg
    x = np.asarray(inp["x"], np.float32)
    token = np.asarray(inp["enc_mask_token"], np.float32).reshape(-1)
    token_zero = not np.any(token)
    tokbf = token.astype(bf16)
    ei = np.asarray(inp["edge_index"])
    src, dst = ei[0].astype(np.int64), ei[1].astype(np.int64)
    batch = np.asarray(inp["batch"]).astype(np.int64)
    mask_nodes = np.asarray(inp["mask_nodes"]).astype(np.int64)
    W = {k: np.asarray(inp[k], np.float32) for k in _WNAMES}

    wtd = np.bincount(dst, minlength=c.N).astype(np.int64)
    wbu = np.bincount(src, minlength=c.N).astype(np.int64)
    dinv = [(1.0 / np.sqrt(wtd + 1.0)).astype(np.float32),
            (1.0 / np.sqrt(wbu + 1.0)).astype(np.float32)]
    mcount = np.bincount(mask_nodes, minlength=c.N).astype(np.float32)
    is_masked = mcount > 0
    xbf = x.astype(bf16)

    core, blk, lane = _assign_nodes(c, wtd, wbu)
    z2row = core * c.OWNP + blk * 128 + lane

    ed = {}
    for d in range(2):
        ad, asr = (dst, src) if d == 0 else (src, dst)
        val = dinv[d][ad] * dinv[d][asr]
        ed[d] = (ad, asr, val)

    all_nodes = np.arange(c.N, dtype=np.int64)
    e_full = {}
    for ci in range(c.C):
        own = core == ci
        on_nodes = all_nodes[own]
        for d in range(2):
            ad, asr, val = ed[d]
            sel = core[ad] == ci
            dl = np.concatenate([lane[ad[sel]], lane[on_nodes]])
            db = np.concatenate([blk[ad[sel]], blk[on_nodes]])
            sr = np.concatenate([asr[sel], on_nodes])
            vv = np.concatenate([val[sel], dinv[d][on_nodes] ** 2])
            vt = vv
            vo = vv * (~is_masked[sr])
            if not token_zero:
                msk = is_masked[sr]
                cacc = np.zeros((c.NB, 128), np.float32)
                np.add.at(cacc, (db[msk], dl[msk]), vv[msk])
                nz = np.nonzero(cacc)
                dl = np.concatenate([dl, nz[1]])
                db = np.concatenate([db, nz[0]])
                sr = np.concatenate([sr, np.full(len(nz[0]), -1, np.int64)])
                vt = np.concatenate([vt, np.zeros(len(nz[0]), np.float32)])
                vo = np.concatenate([vo, cacc[nz]])
            e_full[ci, d] = (dl, db, sr, vt, vo)

    cnt1 = np.zeros((2, c.C, c.NB), np.int64)
    for (ci, d), (dl, db, sr, vt, vo) in e_full.items():
        np.add.at(cnt1[d, ci], db, 1)
    # per-dir processing order: blocks sorted by max-core load so the shared
    # per-position KT profile is tight
    maxl = [cnt1[d].max(axis=0) for d in range(2)]
    ord1 = [np.argsort(-maxl[d], kind="stable").astype(np.int64)
            for d in range(2)]
    pos1 = []
    for d in range(2):
        p = np.empty(c.NB, np.int64)
        p[ord1[d]] = np.arange(c.NB)
        pos1.append(p)
    KT1 = [np.maximum(1, -(-maxl[d][ord1[d]] // 128)) for d in range(2)]
    KT1off = [np.r_[0, np.cumsum(KT1[d])].astype(np.int64) for d in range(2)]
    TOTKT1 = [int(KT1off[d][-1]) for d in range(2)]

    # Band structure: for each (dir) a list per sg:
    #   bandops: ordered [(ko_global, band_idx, blk)]
    # built identically for every core (tile/band LAYOUT is shared; band
    # CONTENT varies per core).  Bands are assigned where ANY core has slots;
    # per-core empty bands ship zero tiles.
    # To keep layout shared, band membership must be core-independent ->
    # derive from per-bucket per-block MAX counts.
    bandmeta = []   # [d] -> dict(nband, per_sg=[(blocks, ops=[(ko,band,blk)],
    #                                nmm={blk:count})])
    cnt2b = np.zeros((2, c.C, c.NG, c.NW2, c.GB1), np.int64)
    for d in range(2):
        ad, asr, _ = ed[d]
        np.add.at(cnt2b[d], (core[ad], blk[ad] // c.GB1,
                             z2row[asr] // c.WIN, blk[ad] % c.GB1), 1)
    # shared per-bucket block layout: use max over cores per (g,w,bi)
    blkmax = [cnt2b[d].max(axis=0) for d in range(2)]   # [NG, NW2, GB1]
    # within bucket, blocks packed in order bi=0..GB1-1, slot ranges from
    # blkmax; bucket capacity = KT2*128 (>= sum blkmax by construction? NO:
    # sum of per-block maxes can exceed 128*KT2). Recompute KT2 from blkmax.
    KT2 = [np.maximum(-(-blkmax[d].sum(axis=2) // 128),
                      (blkmax[d].sum(axis=2) > 0)) for d in range(2)]
    KT2off, sched2, TOTKT2 = [], [], []
    for d in range(2):
        off = np.zeros((c.NG, c.NW2), np.int64)
        acc = 0
        sgs = []
        for sg in range(c.NSG):
            g4s = range(sg * c.SGF, min((sg + 1) * c.SGF, c.NG))
            ops = []
            for w in range(c.NW2):
                nk = int(sum(KT2[d][g, w] for g in g4s))
                if nk == 0:
                    continue
                base = acc
                for g in g4s:
                    off[g, w] = acc
                    acc += KT2[d][g, w]
                ops.append((w, base, nk))
            sgs.append(ops)
        KT2off.append(off)
        sched2.append(sgs)
        TOTKT2.append(int(acc))

    for d in range(2):
        nband = 0
        per_sg = []
        for sg in range(c.NSG):
            g4s = list(range(sg * c.SGF, min((sg + 1) * c.SGF, c.NG)))
            opsl = []
            nmm = {}
            blocks = []
            for g in g4s:
                for bi in range(c.GB1):
                    b = g * c.GB1 + bi
                    if b < c.NB:
                        blocks.append(b)
                        nmm[b] = 1          # self matmul
            for g in g4s:
                for w in range(c.NW2):
                    if KT2[d][g, w] == 0:
                        continue
                    ko = int(KT2off[d][g, w])
                    # slot layout inside bucket: block runs of len blkmax
                    start_s = 0
                    for bi in range(c.GB1):
                        b = g * c.GB1 + bi
                        ln = int(blkmax[d][g, w, bi])
                        if ln == 0 or b >= c.NB:
                            start_s += ln
                            continue
                        t0, t1 = start_s // 128, (start_s + ln - 1) // 128
                        for t in range(t0, t1 + 1):
                            opsl.append((ko + t, nband, b))
                            nband += 1
                            nmm[b] += 1
                        start_s += ln
            per_sg.append(dict(blocks=blocks, ops=opsl, nmm=nmm))
        bandmeta.append(dict(nband=nband, per_sg=per_sg))

    # ---- shared weights etc.
    ident = np.eye(128, dtype=np.float32)
    w1 = {}
    w2 = {}
    b1c = {}
    b2bc = {}
    for d, nm in ((0, "td"), (1, "bu")):
        for v, pre in ((0, "on"), (1, "tgt")):
            wt = W[f"{pre}_{nm}_W1"]
            w1[d, v] = np.ascontiguousarray(
                wt.reshape(4, 128, 128).transpose(1, 0, 2).reshape(128, 512)
            ).astype(bf16)
            w2[d, v] = W[f"{pre}_{nm}_W2"].astype(bf16)
        b1c[d] = np.stack([W[f"on_{nm}_b1"], W[f"tgt_{nm}_b1"]],
                          axis=1).astype(np.float32)
        b2bc[d] = _bcast(np.concatenate([W[f"on_{nm}_b2"],
                                         W[f"tgt_{nm}_b2"]]))
    ones = np.ones((128, 1), np.float32)
    gmask = np.zeros((128, 1), np.float32)
    gmask[:c.G, 0] = 1.0

    # ---- per-core inputs
    in_maps = []
    for ci in range(c.C):
        own_sel = core == ci
        m = {}

        def nodecol(valarr, pad=0.0):
            a = np.full((128, c.NB), pad, np.float32)
            a[lane[own_sel], blk[own_sel]] = valarr[own_sel]
            return a

        m["mw"] = nodecol(mcount)

        # pooling one-hot [node, graph] per block; batch of pad lanes = -1
        bcol = nodecol(batch.astype(np.float32), pad=-1.0)
        pp = np.zeros((c.NB * 128, 128), np.float32)
        pl, pb = lane[own_sel], blk[own_sel]
        pp[pb * 128 + pl, batch[own_sel]] = 1.0
        m["ppool"] = _tile_rows(pp.astype(bf16), 128)

        for d, nm in ((0, "td"), (1, "bu")):
            # self-loop diag tiles
            sfv = nodecol(dinv[d] * dinv[d])
            sd = np.zeros((c.NB * 128, 128), np.float32)
            ll = np.arange(128)
            for b in range(c.NB):
                sd[b * 128 + ll, ll] = sfv[:, b]
            m[f"sd_{nm}"] = _tile_rows(sd.astype(bf16), 128)

            dl, db, sr, vt, vo = e_full[ci, d]
            # --- L1 slots (by processing position) ---
            dpos = pos1[d][db]
            order = np.argsort(dpos, kind="stable")
            sdb, sdl = dpos[order], dl[order]
            ssrc, svt, svo = sr[order], vt[order], vo[order]
            segchange = np.r_[True, sdb[1:] != sdb[:-1]]
            segstart = np.maximum.accumulate(
                np.where(segchange, np.arange(len(sdb)), 0))
            pos = np.arange(len(sdb)) - segstart
            slot = KT1off[d][sdb] * 128 + pos
            nslot1 = TOTKT1[d] * 128
            assert slot.max(initial=0) < nslot1
            xe = np.zeros((nslot1, 512), bf16)
            reg = ssrc >= 0
            xe[slot[reg]] = xbf[ssrc[reg]]
            if not token_zero:
                xe[slot[~reg]] = tokbf
            m[f"xe_{nm}"] = _tile_rows(xe, 512)
            # S tiles: [slot, 256] = [S_on | S_tgt]
            s1 = np.zeros((nslot1, 256), np.float32)
            s1[slot, sdl] = svo
            s1[slot, 128 + sdl] = svt
            m[f"s1_{nm}"] = _tile_rows(s1.astype(bf16), 256)

            # --- L2 slots: bucket (g4, w), block-run layout from blkmax ---
            ad, asr, val = ed[d]
            sel = core[ad] == ci
            f_db = blk[ad[sel]]
            f_g4 = f_db // c.GB1
            f_bi = f_db % c.GB1
            f_lane = lane[ad[sel]]
            f_row = z2row[asr[sel]]
            f_w = f_row // c.WIN
            f_rel = f_row - f_w * c.WIN
            f_val = val[sel]
            # run base offsets within bucket
            runoff = np.zeros((c.NG, c.NW2, c.GB1), np.int64)
            runoff[:, :, 1:] = np.cumsum(blkmax[d], axis=2)[:, :, :-1]
            # position within (g4, w, bi) run
            key = (f_g4 * c.NW2 + f_w) * c.GB1 + f_bi
            order = np.argsort(key, kind="stable")
            kk = key[order]
            segchange = np.r_[True, kk[1:] != kk[:-1]]
            segstart = np.maximum.accumulate(
                np.where(segchange, np.arange(len(kk)), 0))
            pos = np.arange(len(kk)) - segstart
            og4, ow, obi = f_g4[order], f_w[order], f_bi[order]
            slot = (KT2off[d][og4, ow] * 128 +
                    runoff[og4, ow, obi] + pos)
            nslot2 = TOTKT2[d] * 128
            assert slot.max(initial=0) < nslot2
            idx = np.zeros(nslot2, np.int64)
            idx[slot] = f_rel[order]
            m[f"i_{nm}"] = _rep16(idx.astype(np.int16))
            # band S tiles: [128, nband*128]
            bm = bandmeta[d]
            s2 = np.zeros((128, bm["nband"] * 128), np.float32)
            # compute each slot's band: need (ko_tile, block) -> band index
            band_of = {}
            for psg in bm["per_sg"]:
                for (ko, band, b) in psg["ops"]:
                    band_of[ko, b] = band
            okt = slot // 128
            opp = slot % 128
            oband = np.array([band_of[k, g * c.GB1 + bi]
                              for k, g, bi in zip(okt, og4, obi)])
            s2[opp, oband * 128 + f_lane[order]] = f_val[order]
            m[f"s2_{nm}"] = s2.astype(bf16)

            m[f"b2bc_{nm}"] = b2bc[d]
        m.update(ident=ident, ones=ones, gmask=gmask)
        for d, nm in ((0, "td"), (1, "bu")):
            m[f"w1on_{nm}"] = w1[d, 0]
            m[f"w1tg_{nm}"] = w1[d, 1]
            m[f"w2on_{nm}"] = w2[d, 0]
            m[f"w2tg_{nm}"] = w2[d, 1]
            m[f"b1c_{nm}"] = b1c[d]
        in_maps.append(m)

    meta = dict(KT1=KT1, TOTKT1=TOTKT1, KT2=KT2, KT2off=KT2off,
                sched2=sched2, TOTKT2=TOTKT2, bandmeta=bandmeta,
                ord1=ord1, assign=(core, blk, lane), z2row=z2row)
    return meta, in_maps


# ---------------------------------------------------------------- program


def build_program(cfg, meta):
    import concourse.bacc as bacc
    import concourse.mybir as mybir
    import concourse.tile as tile

    c = cfg
    KT1, TOTKT1 = meta["KT1"], meta["TOTKT1"]
    KT2, KT2off, TOTKT2 = meta["KT2"], meta["KT2off"], meta["TOTKT2"]
    sched2, bandmeta = meta["sched2"], meta["bandmeta"]
    f32, bf, i16 = mybir.dt.float32, mybir.dt.bfloat16, mybir.dt.int16
    MUL, ADD, SUB = (mybir.AluOpType.mult, mybir.AluOpType.add,
                     mybir.AluOpType.subtract)
    AF = mybir.ActivationFunctionType

    nc = bacc.Bacc("TRN2", target_bir_lowering=False, debug=False,
                   num_devices=c.C)

    def din(name, shape, dt):
        return nc.dram_tensor(name, shape, dt, kind="ExternalInput")

    DN = ("td", "bu")
    xe_t = [din(f"xe_{n}", [128, TOTKT1[d] * 512], bf) for d, n in enumerate(DN)]
    s1_t = [din(f"s1_{n}", [128, TOTKT1[d] * 256], bf) for d, n in enumerate(DN)]
    i2_t = [din(f"i_{n}", [128, TOTKT2[d] * 8], i16) for d, n in enumerate(DN)]
    s2_t = [din(f"s2_{n}", [128, bandmeta[d]["nband"] * 128], bf)
            for d, n in enumerate(DN)]
    sd_t = [din(f"sd_{n}", [128, c.NB * 128], bf) for d, n in enumerate(DN)]
    pp_t = din("ppool", [128, c.NB * 128], bf)
    mw_t = din("mw", [128, c.NB], f32)
    ident_t = din("ident", [128, 128], f32)
    ones_t = din("ones", [128, 1], f32)
    gmask_t = din("gmask", [128, 1], f32)
    w1on_t = [din(f"w1on_{n}", [128, 512], bf) for n in DN]
    w1tg_t = [din(f"w1tg_{n}", [128, 512], bf) for n in DN]
    w2on_t = [din(f"w2on_{n}", [128, 64], bf) for n in DN]
    w2tg_t = [din(f"w2tg_{n}", [128, 64], bf) for n in DN]
    b1c_t = [din(f"b1c_{n}", [128, 2], f32) for n in DN]
    b2bc_t = [din(f"b2bc_{n}", [128, 128], f32) for n in DN]
    loss_t = nc.dram_tensor("loss", [1, 1], f32, kind="ExternalOutput")

    with tile.TileContext(nc) as tc:
        with (
            tc.tile_pool(name="const", bufs=1) as cpool,
            tc.tile_pool(name="dram", bufs=1, space="DRAM") as dpool,
        ):
            z2own = [dpool.tile([c.OWNP, 128], bf, tag=f"z2own{d}",
                                name=f"z2own{d}") for d in range(2)]
            z2full = [dpool.tile([c.NPAD, 128], bf, addr_space="Shared",
                                 tag=f"z2full{d}", name=f"z2full{d}")
                      for d in range(2)]
            ar_in = dpool.tile([128, 260], f32, tag="arin", name="arin")
            ar_out = dpool.tile([128, 260], f32, addr_space="Shared",
                                tag="arout", name="arout")

            identsb = cpool.tile([128, 128], f32)
            nc.sync.dma_start(out=identsb[:], in_=ident_t[:, :])
            identbf = cpool.tile([128, 128], bf)
            nc.vector.tensor_copy(out=identbf[:], in_=identsb[:])
            onesb = cpool.tile([128, 1], f32)
            nc.sync.dma_start(out=onesb[:], in_=ones_t[:, :])
            gmsb = cpool.tile([128, 1], f32)
            nc.sync.dma_start(out=gmsb[:], in_=gmask_t[:, :])
            mwsb = cpool.tile([128, c.NB], f32)
            nc.sync.dma_start(out=mwsb[:], in_=mw_t[:, :])
            w1sb = [[cpool.tile([128, 512], bf, tag=f"w1_{d}{v}",
                                name=f"w1_{d}{v}") for v in range(2)]
                    for d in range(2)]
            w2sb = [[cpool.tile([128, 64], bf, tag=f"w2_{d}{v}",
                                name=f"w2_{d}{v}") for v in range(2)]
                    for d in range(2)]
            b1csb = [cpool.tile([128, 2], f32, tag=f"b1c_{d}", name=f"b1c_{d}")
                     for d in range(2)]
            b2sb = [cpool.tile([128, 128], f32, tag=f"b2_{d}", name=f"b2_{d}")
                    for d in range(2)]
            for d in range(2):
                nc.sync.dma_start(out=w1sb[d][0][:], in_=w1on_t[d][:, :])
                nc.sync.dma_start(out=w1sb[d][1][:], in_=w1tg_t[d][:, :])
                nc.sync.dma_start(out=w2sb[d][0][:], in_=w2on_t[d][:, :])
                nc.sync.dma_start(out=w2sb[d][1][:], in_=w2tg_t[d][:, :])
                nc.sync.dma_start(out=b1csb[d][:], in_=b1c_t[d][:, :])
                nc.sync.dma_start(out=b2sb[d][:], in_=b2bc_t[d][:, :])

            stash = [cpool.tile([128, c.NB * 128], bf, tag=f"st{d}",
                                name=f"st{d}") for d in range(2)]
            cdot = cpool.tile([128, c.NB], f32)
            cd2 = cpool.tile([128, c.NB], f32)
            cn1 = cpool.tile([128, c.NB], f32)
            cn1b = cpool.tile([128, c.NB], f32)
            cn2 = cpool.tile([128, c.NB], f32)
            cn2b = cpool.tile([128, c.NB], f32)

            # ================= L1 (per dir) ===========================
            def l1_dir(d, pools):
                xep, s1p, fzp, pap, php, ptp = pools
                kt1 = KT1[d]
                ord1 = meta["ord1"][d]
                kt1off = np.r_[0, np.cumsum(kt1)].astype(np.int64)
                if True:
                    for g in range(c.NG):
                        b0 = g * c.GB1
                        blks = list(range(b0, min(b0 + c.GB1, c.NB)))
                        k0, k1 = int(kt1off[b0]), int(kt1off[blks[-1] + 1])
                        nkt = k1 - k0
                        xet = xep.tile([128, nkt * 512], bf, tag="xe",
                                       name="xe")
                        nc.sync.dma_start(
                            out=xet[:], in_=xe_t[d][:, k0 * 512:k1 * 512])
                        s1s = s1p.tile([128, nkt * 256], bf, tag="s1",
                                       name="s1")
                        nc.sync.dma_start(
                            out=s1s[:], in_=s1_t[d][:, k0 * 256:k1 * 256])
                        gw = len(blks) * 128
                        sbA = [fzp.tile([128, 4 * c.GB1 * 128], bf,
                                        tag=f"sbA{v}", name=f"sbA{v}")
                               for v in range(2)]
                        for bi, b in enumerate(blks):
                            psA = [pap.tile([128, 512], f32, tag=f"pA{v}",
                                            name=f"pA{v}") for v in range(2)]
                            for t in range(int(kt1[b])):
                                kt = int(kt1off[b]) + t - k0
                                for v in range(2):
                                    nc.tensor.matmul(
                                        out=psA[v][:],
                                        lhsT=s1s[:, kt * 256 + v * 128:
                                                 kt * 256 + (v + 1) * 128],
                                        rhs=xet[:, kt * 512:(kt + 1) * 512],
                                        start=(t == 0),
                                        stop=(t == int(kt1[b]) - 1))
                            # A -> bf16, transpose chunks, grouped layout
                            for v in range(2):
                                asb = fzp.tile([128, 512], bf, tag=f"as{v}",
                                               name=f"as{v}")
                                nc.scalar.copy(out=asb[:], in_=psA[v][:])
                                trt = ptp.tile([128, 512], bf, tag="tr",
                                               name="tr")
                                for ch in range(4):
                                    nc.tensor.transpose(
                                        out=trt[:, ch * 128:(ch + 1) * 128],
                                        in_=asb[:, ch * 128:(ch + 1) * 128],
                                        identity=identbf[:])
                                nc.vector.tensor_copy(
                                    out=sbA[v][:].rearrange(
                                        "p (ch n) -> p ch n",
                                        ch=4)[:, :, bi * 128:(bi + 1) * 128],
                                    in_=trt[:].rearrange(
                                        "p (ch n) -> p ch n", ch=4))
                        hT = [php.tile([128, c.GB1 * 128], f32, tag=f"hT{v}",
                                       name=f"hT{v}") for v in range(2)]
                        for v in range(2):
                            for ch in range(4):
                                nc.tensor.matmul(
                                    out=hT[v][:, 0:gw],
                                    lhsT=w1sb[d][v][:, ch * 128:(ch + 1) * 128],
                                    rhs=sbA[v][:].rearrange(
                                        "p (ch n) -> p ch n",
                                        ch=4)[:, ch, 0:gw],
                                    start=(ch == 0), stop=(ch == 3))
                        hsb = [fzp.tile([128, c.GB1 * 128], bf, tag=f"h{v}",
                                        name=f"h{v}") for v in range(2)]
                        for v in range(2):
                            nc.scalar.activation(
                                out=hsb[v][:, 0:gw], in_=hT[v][:, 0:gw],
                                func=AF.Relu, bias=b1csb[d][:, v:v + 1])
                        z2T = php.tile([128, c.GB1 * 128], f32, tag="z2T",
                                       name="z2T")
                        for v in range(2):
                            nc.tensor.matmul(
                                out=z2T[v * 64:(v + 1) * 64, 0:gw],
                                lhsT=w2sb[d][v][:], rhs=hsb[v][:, 0:gw],
                                start=True, stop=True)
                        z2Tsb = fzp.tile([128, c.GB1 * 128], bf, tag="z2Tsb",
                                         name="z2Tsb")
                        nc.vector.tensor_copy(out=z2Tsb[:, 0:gw],
                                              in_=z2T[:, 0:gw])
                        trz = ptp.tile([128, c.GB1 * 128], bf, tag="tr",
                                       name="trz")
                        for bi, b in enumerate(blks):
                            nc.tensor.transpose(
                                out=trz[:, bi * 128:(bi + 1) * 128],
                                in_=z2Tsb[:, bi * 128:(bi + 1) * 128],
                                identity=identbf[:])
                        for bi, b in enumerate(blks):
                            ph = int(ord1[b])
                            nc.vector.tensor_copy(
                                out=stash[d][:, ph * 128:(ph + 1) * 128],
                                in_=trz[:, bi * 128:(bi + 1) * 128])
                            nc.sync.dma_start(
                                out=z2own[d][ph * 128:(ph + 1) * 128, :],
                                in_=stash[d][:, ph * 128:(ph + 1) * 128])

            with (
                tc.tile_pool(name="xe", bufs=2) as xep,
                tc.tile_pool(name="s1p", bufs=2) as s1p,
                tc.tile_pool(name="fz", bufs=2) as fzp,
                tc.tile_pool(name="pA", bufs=2, space="PSUM") as pap,
                tc.tile_pool(name="pH", bufs=1, space="PSUM") as php,
                tc.tile_pool(name="pT", bufs=1, space="PSUM") as ptp,
            ):
                pools1 = (xep, s1p, fzp, pap, php, ptp)
                l1_dir(0, pools1)
                nc.gpsimd.collective_compute(
                    "AllGather", mybir.AluOpType.bypass,
                    replica_groups=[list(range(c.C))],
                    ins=[z2own[0].opt()], outs=[z2full[0].opt()])
                l1_dir(1, pools1)
                nc.gpsimd.collective_compute(
                    "AllGather", mybir.AluOpType.bypass,
                    replica_groups=[list(range(c.C))],
                    ins=[z2own[1].opt()], outs=[z2full[1].opt()])

            # ================= L2 (per dir) ===========================
            with tc.tile_pool(name="plps", bufs=1, space="PSUM") as plp:
                pool_ps = [plp.tile([128, 128], f32, tag=f"pl{d}",
                                    name=f"pl{d}") for d in range(2)]

                def l2_dir(d, pools):
                    gzp, ixp, s2p, sdp, fhp, pbp = pools
                    wlen = lambda w: min(c.WIN, c.NPAD - w * c.WIN)
                    pool_mm = [0]
                    tot_pool = c.NB
                    if True:
                        for sg, ops in enumerate(sched2[d]):
                            bm = bandmeta[d]["per_sg"][sg]
                            blocks, bops, nmm = (bm["blocks"], bm["ops"],
                                                 bm["nmm"])
                            if ops:
                                sk0 = ops[0][1]
                                sk1 = ops[-1][1] + ops[-1][2]
                            else:
                                sk0 = sk1 = 0
                            snk = max(sk1 - sk0, 1)
                            if bops:
                                bd0 = bops[0][1]
                                bd1 = bops[-1][1] + 1
                            else:
                                bd0 = bd1 = 0
                            nbd = max(bd1 - bd0, 1)
                            gzt = gzp.tile([128, snk * 128], bf, tag="gz",
                                           name="gz")
                            it = ixp.tile([128, snk * 8], i16, tag="ix",
                                          name="ix")
                            s2s = s2p.tile([128, nbd * 128], bf, tag="s2",
                                           name="s2")
                            sds = sdp.tile([128, len(blocks) * 128], bf,
                                           tag="sd", name="sd")
                            b00 = blocks[0]
                            nc.sync.dma_start(
                                out=sds[:],
                                in_=sd_t[d][:, b00 * 128:
                                            (b00 + len(blocks)) * 128])
                            pps = sdp.tile([128, len(blocks) * 128], bf,
                                           tag="pp", name="pp")
                            nc.sync.dma_start(
                                out=pps[:],
                                in_=pp_t[:, b00 * 128:
                                         (b00 + len(blocks)) * 128])
                            if sk1 > sk0:
                                nc.sync.dma_start(
                                    out=it[:, 0:(sk1 - sk0) * 8],
                                    in_=i2_t[d][:, sk0 * 8:sk1 * 8])
                            if bd1 > bd0:
                                nc.sync.dma_start(
                                    out=s2s[:, 0:(bd1 - bd0) * 128],
                                    in_=s2_t[d][:, bd0 * 128:bd1 * 128])
                            for w, base, nk in ops:
                                o = base - sk0
                                nc.gpsimd.dma_gather(
                                    gzt[:, o * 128:(o + nk) * 128].rearrange(
                                        "p (k e) -> p k e", k=nk, e=128),
                                    z2full[d][w * c.WIN:w * c.WIN + wlen(w), :],
                                    it[:, o * 8:(o + nk) * 8],
                                    nk * 128, nk * 128, 128,
                                    elem_step=None, single_packet=True)
                            # per-block psum slices of 2 group tiles:
                            # self first, then bands
                            psG = [pbp.tile([128, c.GB1 * 128], f32,
                                            tag=f"psG{j}", name=f"psG{j}")
                                   for j in range(c.SGF)]

                            def bslice(b):
                                gi = (b // c.GB1) % c.SGF
                                bi = b % c.GB1
                                return psG[gi][:, bi * 128:(bi + 1) * 128]

                            done = {}
                            for bi, b in enumerate(blocks):
                                nc.tensor.matmul(
                                    out=bslice(b),
                                    lhsT=sds[:, bi * 128:(bi + 1) * 128],
                                    rhs=stash[d][:, b * 128:(b + 1) * 128],
                                    start=True, stop=(nmm[b] == 1),
                                    skip_group_check=True)
                                done[b] = 1
                            for (ko, band, b) in bops:
                                nc.tensor.matmul(
                                    out=bslice(b),
                                    lhsT=s2s[:, (band - bd0) * 128:
                                             (band - bd0 + 1) * 128],
                                    rhs=gzt[:, (ko - sk0) * 128:
                                            (ko - sk0 + 1) * 128],
                                    start=False,
                                    stop=(done[b] + 1 == nmm[b]),
                                    skip_group_check=True)
                                done[b] += 1
                            # finalize per block
                            for bi, b in enumerate(blocks):
                                h2b = fhp.tile([128, 128], bf, tag="h2b",
                                               name="h2b")
                                nc.vector.tensor_tensor(
                                    out=h2b[:], in0=bslice(b),
                                    in1=b2sb[d][:], op=ADD)
                                nc.tensor.matmul(
                                    out=pool_ps[d][:],
                                    lhsT=pps[:, bi * 128:(bi + 1) * 128],
                                    rhs=h2b[:],
                                    start=(pool_mm[0] == 0),
                                    stop=(pool_mm[0] == tot_pool - 1),
                                    skip_group_check=True)
                                pool_mm[0] += 1
                                if d == 0:
                                    nc.vector.tensor_copy(
                                        out=stash[0][:, b * 128:(b + 1) * 128],
                                        in_=h2b[:])
                                else:
                                    td = stash[0][:, b * 128:(b + 1) * 128]
                                    sc = fhp.tile([128, 64], f32, tag="sc",
                                                  name="sc")
                                    sc2 = fhp.tile([128, 64], f32, tag="sc2",
                                                   name="sc2")
                                    nc.vector.scalar_tensor_tensor(
                                        out=sc[:], in0=td[:, 0:64], scalar=1.0,
                                        in1=td[:, 64:128], op0=MUL, op1=MUL,
                                        accum_out=cdot[:, b:b + 1])
                                    nc.vector.scalar_tensor_tensor(
                                        out=sc2[:], in0=h2b[:, 0:64],
                                        scalar=1.0, in1=h2b[:, 64:128],
                                        op0=MUL, op1=MUL,
                                        accum_out=cd2[:, b:b + 1])
                                    nc.vector.scalar_tensor_tensor(
                                        out=sc[:], in0=td[:, 0:64], scalar=1.0,
                                        in1=td[:, 0:64], op0=MUL, op1=MUL,
                                        accum_out=cn1[:, b:b + 1])
                                    nc.vector.scalar_tensor_tensor(
                                        out=sc2[:], in0=h2b[:, 0:64],
                                        scalar=1.0, in1=h2b[:, 0:64],
                                        op0=MUL, op1=MUL,
                                        accum_out=cn1b[:, b:b + 1])
                                    nc.vector.scalar_tensor_tensor(
                                        out=sc[:], in0=td[:, 64:128],
                                        scalar=1.0, in1=td[:, 64:128],
                                        op0=MUL, op1=MUL,
                                        accum_out=cn2[:, b:b + 1])
                                    nc.vector.scalar_tensor_tensor(
                                        out=sc2[:], in0=h2b[:, 64:128],
                                        scalar=1.0, in1=h2b[:, 64:128],
                                        op0=MUL, op1=MUL,
                                        accum_out=cn2b[:, b:b + 1])

                with (
                    tc.tile_pool(name="gz", bufs=2) as gzp,
                    tc.tile_pool(name="ix", bufs=2) as ixp,
                    tc.tile_pool(name="s2p", bufs=2) as s2p,
                    tc.tile_pool(name="sdp", bufs=2) as sdp,
                    tc.tile_pool(name="fh", bufs=3) as fhp,
                    tc.tile_pool(name="pB", bufs=2, space="PSUM") as pbp,
                ):
                    pools2 = (gzp, ixp, s2p, sdp, fhp, pbp)
                    l2_dir(0, pools2)
                    l2_dir(1, pools2)

                # ============ masked cosine tail + pool AR ============
                with (
                    tc.tile_pool(name="tail", bufs=2) as tlp,
                    tc.tile_pool(name="tps", bufs=2, space="PSUM") as tpp,
                ):
                    dot = tlp.tile([128, c.NB], f32, tag="dot", name="dot")
                    nc.vector.tensor_tensor(out=dot[:], in0=cdot[:],
                                            in1=cd2[:], op=ADD)
                    n1 = tlp.tile([128, c.NB], f32, tag="n1", name="n1")
                    nc.vector.tensor_tensor(out=n1[:], in0=cn1[:], in1=cn1b[:],
                                            op=ADD)
                    n2 = tlp.tile([128, c.NB], f32, tag="n2", name="n2")
                    nc.vector.tensor_tensor(out=n2[:], in0=cn2[:], in1=cn2b[:],
                                            op=ADD)

                    def rcp_sqrt(nt, tag):
                        r = tlp.tile([128, c.NB], f32, tag=tag, name=tag)
                        nc.scalar.sqrt(out=r[:], in_=nt[:])
                        nc.vector.tensor_scalar_max(out=r[:], in0=r[:],
                                                    scalar1=1e-12)
                        nc.vector.reciprocal(out=r[:], in_=r[:])
                        return r

                    r1 = rcp_sqrt(n1, "r1")
                    r2 = rcp_sqrt(n2, "r2")
                    cosv = tlp.tile([128, c.NB], f32, tag="cosv", name="cosv")
                    nc.vector.tensor_tensor(out=cosv[:], in0=dot[:], in1=r1[:],
                                            op=MUL)
                    nc.vector.tensor_tensor(out=cosv[:], in0=cosv[:],
                                            in1=r2[:], op=MUL)
                    term = tlp.tile([128, c.NB], f32, tag="term", name="term")
                    macc = tlp.tile([128, 1], f32, tag="macc", name="macc")
                    nc.vector.tensor_tensor(out=term[:], in0=mwsb[:],
                                            in1=cosv[:], op=MUL)
                    nc.vector.scalar_tensor_tensor(
                        out=term[:], in0=mwsb[:], scalar=1.0, in1=term[:],
                        op0=MUL, op1=SUB, accum_out=macc[:])
                    msps = tpp.tile([1, 1], f32, tag="ms", name="ms")
                    nc.tensor.matmul(out=msps[:], lhsT=macc[:], rhs=onesb[:],
                                     start=True, stop=True)

                    arsb = tlp.tile([128, 260], f32, tag="arsb", name="arsb")
                    nc.vector.memset(arsb[:], 0.0)
                    nc.vector.tensor_copy(out=arsb[:, 0:128],
                                          in_=pool_ps[0][:])
                    nc.vector.tensor_copy(out=arsb[:, 128:256],
                                          in_=pool_ps[1][:])
                    nc.vector.tensor_copy(out=arsb[0:1, 256:257], in_=msps[:])
                    nc.sync.dma_start(out=ar_in[:, :], in_=arsb[:])
                    nc.gpsimd.collective_compute(
                        "AllReduce", mybir.AluOpType.add,
                        replica_groups=[list(range(c.C))],
                        ins=[ar_in.opt()], outs=[ar_out.opt()])
                    ar2 = tlp.tile([128, 260], f32, tag="ar2", name="ar2")
                    nc.sync.dma_start(out=ar2[:], in_=ar_out[:, :])

                    def gacc(u0, u1, v0, v1, tag):
                        a1 = tlp.tile([128, 1], f32, tag=f"{tag}a",
                                      name=f"{tag}a")
                        a2 = tlp.tile([128, 1], f32, tag=f"{tag}b",
                                      name=f"{tag}b")
                        scr = tlp.tile([128, 64], f32, tag=f"{tag}s",
                                       name=f"{tag}s")
                        nc.vector.scalar_tensor_tensor(
                            out=scr[:], in0=u0, scalar=1.0, in1=v0,
                            op0=MUL, op1=MUL, accum_out=a1[:])
                        nc.vector.scalar_tensor_tensor(
                            out=scr[:], in0=u1, scalar=1.0, in1=v1,
                            op0=MUL, op1=MUL, accum_out=a2[:])
                        s = tlp.tile([128, 1], f32, tag=f"{tag}c",
                                     name=f"{tag}c")
                        nc.vector.tensor_tensor(out=s[:], in0=a1[:],
                                                in1=a2[:], op=ADD)
                        return s

                    tdon, tdtg = ar2[:, 0:64], ar2[:, 64:128]
                    buon, butg = ar2[:, 128:192], ar2[:, 192:256]
                    gdot = gacc(tdon, buon, tdtg, butg, "gd")
                    gn1 = gacc(tdon, buon, tdon, buon, "g1")
                    gn2 = gacc(tdtg, butg, tdtg, butg, "g2")

                    def rcp1(nt, tag):
                        r = tlp.tile([128, 1], f32, tag=tag, name=tag)
                        nc.scalar.sqrt(out=r[:], in_=nt[:])
                        nc.vector.tensor_scalar_max(out=r[:], in0=r[:],
                                                    scalar1=1e-12)
                        nc.vector.reciprocal(out=r[:], in_=r[:])
                        return r

                    gr1 = rcp1(gn1, "gr1")
                    gr2 = rcp1(gn2, "gr2")
                    cosg = tlp.tile([128, 1], f32, tag="cosg", name="cosg")
                    nc.vector.tensor_tensor(out=cosg[:], in0=gdot[:],
                                            in1=gr1[:], op=MUL)
                    nc.vector.tensor_tensor(out=cosg[:], in0=cosg[:],
                                            in1=gr2[:], op=MUL)
                    gterm = tlp.tile([128, 1], f32, tag="gt", name="gt")
                    nc.vector.tensor_scalar(out=gterm[:], in0=cosg[:],
                                            scalar1=-1.0, scalar2=1.0,
                                            op0=MUL, op1=ADD)
                    nc.vector.tensor_tensor(out=gterm[:], in0=gterm[:],
                                            in1=gmsb[:], op=MUL)
                    gsps = tpp.tile([1, 1], f32, tag="gs", name="gs")
                    nc.tensor.matmul(out=gsps[:], lhsT=gterm[:], rhs=onesb[:],
                                     start=True, stop=True)
                    l1t = tlp.tile([1, 1], f32, tag="l1", name="l1")
                    nc.scalar.activation(out=l1t[:], in_=gsps[:], func=AF.Copy,
                                         scale=1.0 / c.G)
                    l2t = tlp.tile([1, 1], f32, tag="l2", name="l2")
                    nc.scalar.activation(out=l2t[:], in_=ar2[0:1, 256:257],
                                         func=AF.Copy, scale=1.0 / c.M)
                    nc.vector.tensor_tensor(out=l1t[:], in0=l1t[:],
                                            in1=l2t[:], op=ADD)
                    nc.sync.dma_start(out=loss_t[:, :], in_=l1t[:])

    return nc


# ---------------------------------------------------------------- entry

LAST_RESULT = None


def kernel(_trace=False, **inputs):
    global LAST_RESULT
    import time
    from concourse import bass_utils
    cfg = FULL
    t0 = time.monotonic()
    meta, in_maps = host_prep(cfg, inputs)
    t1 = time.monotonic()
    nc = build_program(cfg, meta)
    t2 = time.monotonic()
    nc.compile()
    t3 = time.monotonic()
    res = bass_utils.run_bass_kernel_spmd(
        nc, in_maps, core_ids=list(range(cfg.C)),
        trace=_trace, trace_cores=[0] if _trace else None)
    t4 = time.monotonic()
    print(f"[kernel] prep {t1-t0:.1f}s build {t2-t1:.1f}s "
          f"compile {t3-t2:.1f}s run {t4-t3:.1f}s", file=sys.stderr)
    LAST_RESULT = res
    return np.float32(res.results[0]["loss"][0, 0])


# revision 30
# speedup vs baseline: 1.0538x; 1.0538x over previous
"""Trainium2 Bass kernel for the rumor-GCN masked-autoencoder loss (v4).

Strategy (8 NeuronCores, SPMD single NEFF):
  - Layer 1 message passing is HOST-PREGATHERED: for each core/direction the
    host materializes xe[slot] = x[src[slot]] in (dst-block, k-tile) slot
    order, shipped pre-tiled [128, KT*512] bf16, streamed with large
    sequential DMAs -- zero gather descriptors for layer 1.
  - All one-hot S matrices (GCN norms dinv[src]*dinv[dst] folded into the
    values; "on"-encoder masking folded by zeroing masked-src rows) are
    PRE-BUILT ON THE HOST and streamed as bf16 -- no on-chip generation.
  - L1 aggregation in normal form: A[node, 512] += S_v^T @ xe per k-tile
    (two wide matmuls); A transposed chunkwise on the PE; W1/W2 applied per
    4-block group with wide matmuls in feature-major form; z2 transposed
    back per block for the AllGather and stashed in SBUF.
  - Layer 2: z2 AllGathered (bf16); dma_gather per (8-block supergroup, 32K
    window) with (4-block group, window) k-tile buckets; aggregation with
    host-shipped narrow per-(tile, block) band S tiles into per-block
    PSUM accumulators (node-major, no transposes); self-loop terms from the
    SBUF z2 stash via host-shipped diag tiles (no gather slots).
  - Pooling via host-shipped batch-onehot matmuls into persistent PSUM; the
    masked-node cosine loss is computed per block inside the L2-BU loop
    (wide-tile tail), one small AllReduce finishes the scalar loss.
"""

import sys

import numpy as np

sys.path.insert(0, "/opt/trn_rl_repo")

# ---------------------------------------------------------------- config


class Cfg:
    def __init__(self, N, E, G, M, C=8, WIN=28672, GB1=4, SGF=2):
        self.N, self.E, self.G, self.M, self.C = N, E, G, M, C
        self.IN, self.HID, self.OUT = 512, 128, 64
        self.WIN = WIN
        self.GB1 = GB1          # blocks per psum group
        self.SGF = SGF          # L2: psum groups per gather supergroup
        assert N % C == 0
        self.OWN = N // C
        self.NB = -(-self.OWN // 128)
        self.OWNP = self.NB * 128
        self.NPAD = C * self.OWNP
        self.NW2 = -(-self.NPAD // WIN)
        self.NG = -(-self.NB // GB1)
        self.NSG = -(-self.NG // SGF)


FULL = Cfg(N=200000, E=400000, G=128, M=100000)

_WNAMES = [p + s for p in ("on_td", "on_bu", "tgt_td", "tgt_bu")
           for s in ("_W1", "_b1", "_W2", "_b2")]


def _rep16(idx_flat):
    n = len(idx_flat)
    assert n % 16 == 0
    blk = np.zeros((16, n // 16), dtype=np.int16)
    k = np.arange(n)
    blk[k % 16, k // 16] = idx_flat
    return np.tile(blk, (8, 1))


def _bcast(vec, parts=128):
    return np.broadcast_to(np.asarray(vec, np.float32)[None, :],
                           (parts, len(vec))).copy()


def _tile_rows(a, ncols):
    kt = a.shape[0] // 128
    return np.ascontiguousarray(
        a.reshape(kt, 128, ncols).transpose(1, 0, 2).reshape(128, kt * ncols))


# ---------------------------------------------------------------- host prep


def _assign_nodes(cfg, wtd, wbu):
    """Node -> (core, block, lane): snake-deal to cores, then capped greedy
    packing into blocks so per-(block, dir) slot loads stay <= CAP (=3
    k-tiles) for nearly every block."""
    import heapq
    c = cfg
    CAP = 384
    w = wtd + wbu
    order = np.argsort(-w, kind="stable")
    core = np.empty(c.N, np.int64)
    grid = np.arange(c.N) % (2 * c.C)
    snake = np.where(grid < c.C, grid, 2 * c.C - 1 - grid)
    core[order] = snake
    blk = np.empty(c.N, np.int64)
    lane = np.empty(c.N, np.int64)
    std = wtd + 1
    sbu = wbu + 1
    for ci in range(c.C):
        nodes = order[core[order] == ci]      # weight-desc
        ltd = [0] * c.NB
        lbu = [0] * c.NB
        cnt = [0] * c.NB
        heap = [(0, b) for b in range(c.NB)]
        heapq.heapify(heap)
        for v in nodes:
            a, b_ = int(std[v]), int(sbu[v])
            popped = []
            pick = None
            while heap:
                key, b = heapq.heappop(heap)
                if key != ltd[b] + lbu[b]:
                    continue                  # stale entry
                popped.append(b)
                if (cnt[b] < 128 and ltd[b] + a <= CAP
                        and lbu[b] + b_ <= CAP):
                    pick = b
                    break
                if len(popped) >= 12:
                    break
            if pick is None:                  # spill: least-loaded with room
                for b in popped:
                    if cnt[b] < 128:
                        pick = b
                        break
            if pick is None:
                pick = min(range(c.NB), key=lambda x: (cnt[x] >= 128,
                                                       ltd[x] + lbu[x]))
            blk[v] = pick
            lane[v] = cnt[pick]
            cnt[pick] += 1
            ltd[pick] += a
            lbu[pick] += b_
            for b in popped:
                if cnt[b] < 128:
                    heapq.heappush(heap, (ltd[b] + lbu[b], b))
    assert lane.max() < 128
    return core, blk, lane


def host_prep(cfg, inp):
    import ml_dtypes
    bf16 = ml_dtypes.bfloat16
    c = cfg
    x = np.asarray(inp["x"], np.float32)
    token = np.asarray(inp["enc_mask_token"], np.float32).reshape(-1)
    token_zero = not np.any(token)
    tokbf = token.astype(bf16)
    ei = np.asarray(inp["edge_index"])
    src, dst = ei[0].astype(np.int64), ei[1].astype(np.int64)
    batch = np.asarray(inp["batch"]).astype(np.int64)
    mask_nodes = np.asarray(inp["mask_nodes"]).astype(np.int64)
    W = {k: np.asarray(inp[k], np.float32) for k in _WNAMES}

    wtd = np.bincount(dst, minlength=c.N).astype(np.int64)
    wbu = np.bincount(src, minlength=c.N).astype(np.int64)
    dinv = [(1.0 / np.sqrt(wtd + 1.0)).astype(np.float32),
            (1.0 / np.sqrt(wbu + 1.0)).astype(np.float32)]
    mcount = np.bincount(mask_nodes, minlength=c.N).astype(np.float32)
    is_masked = mcount > 0
    xbf = x.astype(bf16)

    core, blk, lane = _assign_nodes(c, wtd, wbu)
    z2row = core * c.OWNP + blk * 128 + lane

    ed = {}
    for d in range(2):
        ad, asr = (dst, src) if d == 0 else (src, dst)
        val = dinv[d][ad] * dinv[d][asr]
        ed[d] = (ad, asr, val)

    all_nodes = np.arange(c.N, dtype=np.int64)
    e_full = {}
    for ci in range(c.C):
        own = core == ci
        on_nodes = all_nodes[own]
        for d in range(2):
            ad, asr, val = ed[d]
            sel = core[ad] == ci
            dl = np.concatenate([lane[ad[sel]], lane[on_nodes]])
            db = np.concatenate([blk[ad[sel]], blk[on_nodes]])
            sr = np.concatenate([asr[sel], on_nodes])
            vv = np.concatenate([val[sel], dinv[d][on_nodes] ** 2])
            vt = vv
            vo = vv * (~is_masked[sr])
            if not token_zero:
                msk = is_masked[sr]
                cacc = np.zeros((c.NB, 128), np.float32)
                np.add.at(cacc, (db[msk], dl[msk]), vv[msk])
                nz = np.nonzero(cacc)
                dl = np.concatenate([dl, nz[1]])
                db = np.concatenate([db, nz[0]])
                sr = np.concatenate([sr, np.full(len(nz[0]), -1, np.int64)])
                vt = np.concatenate([vt, np.zeros(len(nz[0]), np.float32)])
                vo = np.concatenate([vo, cacc[nz]])
            # section: 0 = unmasked src (on+tgt), 1 = masked src (tgt only),
            # 2 = token slots (on only)
            sec = np.where(sr < 0, 2,
                           is_masked[np.maximum(sr, 0)].astype(np.int64))
            e_full[ci, d] = (dl, db, sr, vt, vo, sec)

    NSEC = 2 if token_zero else 3
    cnt1 = np.zeros((2, c.C, c.NB, NSEC), np.int64)
    for (ci, d), (dl, db, sr, vt, vo, sec) in e_full.items():
        np.add.at(cnt1[d, ci], (db, sec), 1)
    # per-dir processing order: blocks sorted by max-core total load
    maxsec = [cnt1[d].max(axis=0) for d in range(2)]        # [NB, NSEC]
    maxl = [maxsec[d].sum(axis=1) for d in range(2)]
    ord1 = [np.argsort(-maxl[d], kind="stable").astype(np.int64)
            for d in range(2)]
    pos1 = []
    for d in range(2):
        p = np.empty(c.NB, np.int64)
        p[ord1[d]] = np.arange(c.NB)
        pos1.append(p)
    # per-position per-section tile counts
    KT1S = [-(-maxsec[d][ord1[d]] // 128) for d in range(2)]  # [pos, NSEC]
    KT1 = [np.maximum(1, KT1S[d].sum(axis=1)) for d in range(2)]
    KT1off = [np.r_[0, np.cumsum(KT1[d])].astype(np.int64) for d in range(2)]
    # section tile offsets within each position
    SECOFF = [np.concatenate([np.zeros((c.NB, 1), np.int64),
                              np.cumsum(KT1S[d], axis=1)[:, :-1]], axis=1)
              for d in range(2)]
    TOTKT1 = [int(KT1off[d][-1]) for d in range(2)]

    # Band structure: for each (dir) a list per sg:
    #   bandops: ordered [(ko_global, band_idx, blk)]
    # built identically for every core (tile/band LAYOUT is shared; band
    # CONTENT varies per core).  Bands are assigned where ANY core has slots;
    # per-core empty bands ship zero tiles.
    # To keep layout shared, band membership must be core-independent ->
    # derive from per-bucket per-block MAX counts.
    bandmeta = []   # [d] -> dict(nband, per_sg=[(blocks, ops=[(ko,band,blk)],
    #                                nmm={blk:count})])
    cnt2b = np.zeros((2, c.C, c.NG, c.NW2, c.GB1), np.int64)
    for d in range(2):
        ad, asr, _ = ed[d]
        np.add.at(cnt2b[d], (core[ad], blk[ad] // c.GB1,
                             z2row[asr] // c.WIN, blk[ad] % c.GB1), 1)
    # shared per-bucket block layout: use max over cores per (g,w,bi)
    blkmax = [cnt2b[d].max(axis=0) for d in range(2)]   # [NG, NW2, GB1]
    # within bucket, blocks packed in order bi=0..GB1-1, slot ranges from
    # blkmax; bucket capacity = KT2*128 (>= sum blkmax by construction? NO:
    # sum of per-block maxes can exceed 128*KT2). Recompute KT2 from blkmax.
    KT2 = [np.maximum(-(-blkmax[d].sum(axis=2) // 128),
                      (blkmax[d].sum(axis=2) > 0)) for d in range(2)]
    KT2off, sched2, TOTKT2 = [], [], []
    for d in range(2):
        off = np.zeros((c.NG, c.NW2), np.int64)
        acc = 0
        sgs = []
        for sg in range(c.NSG):
            g4s = range(sg * c.SGF, min((sg + 1) * c.SGF, c.NG))
            ops = []
            for w in range(c.NW2):
                nk = int(sum(KT2[d][g, w] for g in g4s))
                if nk == 0:
                    continue
                base = acc
                for g in g4s:
                    off[g, w] = acc
                    acc += KT2[d][g, w]
                ops.append((w, base, nk))
            sgs.append(ops)
        KT2off.append(off)
        sched2.append(sgs)
        TOTKT2.append(int(acc))

    for d in range(2):
        nband = 0
        per_sg = []
        for sg in range(c.NSG):
            g4s = list(range(sg * c.SGF, min((sg + 1) * c.SGF, c.NG)))
            opsl = []
            nmm = {}
            blocks = []
            for g in g4s:
                for bi in range(c.GB1):
                    b = g * c.GB1 + bi
                    if b < c.NB:
                        blocks.append(b)
                        nmm[b] = 1          # self matmul
            for g in g4s:
                for w in range(c.NW2):
                    if KT2[d][g, w] == 0:
                        continue
                    ko = int(KT2off[d][g, w])
                    # slot layout inside bucket: block runs of len blkmax
                    start_s = 0
                    for bi in range(c.GB1):
                        b = g * c.GB1 + bi
                        ln = int(blkmax[d][g, w, bi])
                        if ln == 0 or b >= c.NB:
                            start_s += ln
                            continue
                        t0, t1 = start_s // 128, (start_s + ln - 1) // 128
                        for t in range(t0, t1 + 1):
                            opsl.append((ko + t, nband, b))
                            nband += 1
                            nmm[b] += 1
                        start_s += ln
            per_sg.append(dict(blocks=blocks, ops=opsl, nmm=nmm))
        bandmeta.append(dict(nband=nband, per_sg=per_sg))

    # ---- shared weights etc.
    ident = np.eye(128, dtype=np.float32)
    w1 = {}
    w2 = {}
    b1c = {}
    b2bc = {}
    for d, nm in ((0, "td"), (1, "bu")):
        for v, pre in ((0, "on"), (1, "tgt")):
            wt = W[f"{pre}_{nm}_W1"]
            w1[d, v] = np.ascontiguousarray(
                wt.reshape(4, 128, 128).transpose(1, 0, 2).reshape(128, 512)
            ).astype(bf16)
            w2[d, v] = W[f"{pre}_{nm}_W2"].astype(bf16)
        b1c[d] = np.stack([W[f"on_{nm}_b1"], W[f"tgt_{nm}_b1"]],
                          axis=1).astype(np.float32)
        b2bc[d] = _bcast(np.concatenate([W[f"on_{nm}_b2"],
                                         W[f"tgt_{nm}_b2"]]))
    ones = np.ones((128, 1), np.float32)
    gmask = np.zeros((128, 1), np.float32)
    gmask[:c.G, 0] = 1.0

    # ---- per-core inputs
    in_maps = []
    for ci in range(c.C):
        own_sel = core == ci
        m = {}

        def nodecol(valarr, pad=0.0):
            a = np.full((128, c.NB), pad, np.float32)
            a[lane[own_sel], blk[own_sel]] = valarr[own_sel]
            return a

        m["mw"] = nodecol(mcount)

        # pooling one-hot [node, graph] per block; batch of pad lanes = -1
        bcol = nodecol(batch.astype(np.float32), pad=-1.0)
        pp = np.zeros((c.NB * 128, 128), np.float32)
        pl, pb = lane[own_sel], blk[own_sel]
        pp[pb * 128 + pl, batch[own_sel]] = 1.0
        m["ppool"] = _tile_rows(pp.astype(bf16), 128)

        for d, nm in ((0, "td"), (1, "bu")):
            # self-loop diag tiles
            sfv = nodecol(dinv[d] * dinv[d])
            sd = np.zeros((c.NB * 128, 128), np.float32)
            ll = np.arange(128)
            for b in range(c.NB):
                sd[b * 128 + ll, ll] = sfv[:, b]
            m[f"sd_{nm}"] = _tile_rows(sd.astype(bf16), 128)

            dl, db, sr, vt, vo, esec = e_full[ci, d]
            # --- L1 slots (by processing position, then section) ---
            dpos = pos1[d][db]
            skey = dpos * 4 + esec
            order = np.argsort(skey, kind="stable")
            sdb, sdl = dpos[order], dl[order]
            ssec = esec[order]
            ssrc, svt, svo = sr[order], vt[order], vo[order]
            kk = skey[order]
            segchange = np.r_[True, kk[1:] != kk[:-1]]
            segstart = np.maximum.accumulate(
                np.where(segchange, np.arange(len(kk)), 0))
            pos = np.arange(len(kk)) - segstart
            slot = (KT1off[d][sdb] + SECOFF[d][sdb, ssec]) * 128 + pos
            nslot1 = TOTKT1[d] * 128
            assert slot.max(initial=0) < nslot1
            xe = np.zeros((nslot1, 512), bf16)
            reg = ssrc >= 0
            xe[slot[reg]] = xbf[ssrc[reg]]
            if not token_zero:
                xe[slot[~reg]] = tokbf
            m[f"xe_{nm}"] = _tile_rows(xe, 512)
            # single-variant S tiles: value = tgt val (sections 0/1) or the
            # token "on" weight (section 2)
            s1 = np.zeros((nslot1, 128), np.float32)
            sval = np.where(ssec == 2, svo, svt)
            s1[slot, sdl] = sval
            m[f"s1_{nm}"] = _tile_rows(s1.astype(bf16), 128)

            # --- L2 slots: bucket (g4, w), block-run layout from blkmax ---
            ad, asr, val = ed[d]
            sel = core[ad] == ci
            f_db = blk[ad[sel]]
            f_g4 = f_db // c.GB1
            f_bi = f_db % c.GB1
            f_lane = lane[ad[sel]]
            f_row = z2row[asr[sel]]
            f_w = f_row // c.WIN
            f_rel = f_row - f_w * c.WIN
            f_val = val[sel]
            # run base offsets within bucket
            runoff = np.zeros((c.NG, c.NW2, c.GB1), np.int64)
            runoff[:, :, 1:] = np.cumsum(blkmax[d], axis=2)[:, :, :-1]
            # position within (g4, w, bi) run
            key = (f_g4 * c.NW2 + f_w) * c.GB1 + f_bi
            order = np.argsort(key, kind="stable")
            kk = key[order]
            segchange = np.r_[True, kk[1:] != kk[:-1]]
            segstart = np.maximum.accumulate(
                np.where(segchange, np.arange(len(kk)), 0))
            pos = np.arange(len(kk)) - segstart
            og4, ow, obi = f_g4[order], f_w[order], f_bi[order]
            slot = (KT2off[d][og4, ow] * 128 +
                    runoff[og4, ow, obi] + pos)
            nslot2 = TOTKT2[d] * 128
            assert slot.max(initial=0) < nslot2
            idx = np.zeros(nslot2, np.int64)
            idx[slot] = f_rel[order]
            m[f"i_{nm}"] = _rep16(idx.astype(np.int16))
            # band S tiles: [128, nband*128]
            bm = bandmeta[d]
            s2 = np.zeros((128, bm["nband"] * 128), np.float32)
            # compute each slot's band: need (ko_tile, block) -> band index
            band_of = {}
            for psg in bm["per_sg"]:
                for (ko, band, b) in psg["ops"]:
                    band_of[ko, b] = band
            okt = slot // 128
            opp = slot % 128
            oband = np.array([band_of[k, g * c.GB1 + bi]
                              for k, g, bi in zip(okt, og4, obi)])
            s2[opp, oband * 128 + f_lane[order]] = f_val[order]
            m[f"s2_{nm}"] = s2.astype(bf16)

            m[f"b2bc_{nm}"] = b2bc[d]
        m.update(ident=ident, ones=ones, gmask=gmask)
        for d, nm in ((0, "td"), (1, "bu")):
            m[f"w1on_{nm}"] = w1[d, 0]
            m[f"w1tg_{nm}"] = w1[d, 1]
            m[f"w2on_{nm}"] = w2[d, 0]
            m[f"w2tg_{nm}"] = w2[d, 1]
            m[f"b1c_{nm}"] = b1c[d]
        in_maps.append(m)

    meta = dict(KT1=KT1, KT1S=KT1S, SECOFF=SECOFF, NSEC=NSEC,
                TOTKT1=TOTKT1, KT2=KT2, KT2off=KT2off,
                sched2=sched2, TOTKT2=TOTKT2, bandmeta=bandmeta,
                ord1=ord1, assign=(core, blk, lane), z2row=z2row)
    return meta, in_maps


# ---------------------------------------------------------------- program


def build_program(cfg, meta):
    import concourse.bacc as bacc
    import concourse.mybir as mybir
    import concourse.tile as tile

    c = cfg
    KT1, TOTKT1 = meta["KT1"], meta["TOTKT1"]
    KT2, KT2off, TOTKT2 = meta["KT2"], meta["KT2off"], meta["TOTKT2"]
    sched2, bandmeta = meta["sched2"], meta["bandmeta"]
    f32, bf, i16 = mybir.dt.float32, mybir.dt.bfloat16, mybir.dt.int16
    MUL, ADD, SUB = (mybir.AluOpType.mult, mybir.AluOpType.add,
                     mybir.AluOpType.subtract)
    AF = mybir.ActivationFunctionType

    nc = bacc.Bacc("TRN2", target_bir_lowering=False, debug=False,
                   num_devices=c.C)

    def din(name, shape, dt):
        return nc.dram_tensor(name, shape, dt, kind="ExternalInput")

    DN = ("td", "bu")
    xe_t = [din(f"xe_{n}", [128, TOTKT1[d] * 512], bf) for d, n in enumerate(DN)]
    s1_t = [din(f"s1_{n}", [128, TOTKT1[d] * 128], bf) for d, n in enumerate(DN)]
    i2_t = [din(f"i_{n}", [128, TOTKT2[d] * 8], i16) for d, n in enumerate(DN)]
    s2_t = [din(f"s2_{n}", [128, bandmeta[d]["nband"] * 128], bf)
            for d, n in enumerate(DN)]
    sd_t = [din(f"sd_{n}", [128, c.NB * 128], bf) for d, n in enumerate(DN)]
    pp_t = din("ppool", [128, c.NB * 128], bf)
    mw_t = din("mw", [128, c.NB], f32)
    ident_t = din("ident", [128, 128], f32)
    ones_t = din("ones", [128, 1], f32)
    gmask_t = din("gmask", [128, 1], f32)
    w1on_t = [din(f"w1on_{n}", [128, 512], bf) for n in DN]
    w1tg_t = [din(f"w1tg_{n}", [128, 512], bf) for n in DN]
    w2on_t = [din(f"w2on_{n}", [128, 64], bf) for n in DN]
    w2tg_t = [din(f"w2tg_{n}", [128, 64], bf) for n in DN]
    b1c_t = [din(f"b1c_{n}", [128, 2], f32) for n in DN]
    b2bc_t = [din(f"b2bc_{n}", [128, 128], f32) for n in DN]
    loss_t = nc.dram_tensor("loss", [1, 1], f32, kind="ExternalOutput")

    with tile.TileContext(nc) as tc:
        with (
            tc.tile_pool(name="const", bufs=1) as cpool,
            tc.tile_pool(name="dram", bufs=1, space="DRAM") as dpool,
        ):
            z2own = [dpool.tile([c.OWNP, 128], bf, tag=f"z2own{d}",
                                name=f"z2own{d}") for d in range(2)]
            z2full = [dpool.tile([c.NPAD, 128], bf, addr_space="Shared",
                                 tag=f"z2full{d}", name=f"z2full{d}")
                      for d in range(2)]
            ar_in = dpool.tile([128, 260], f32, tag="arin", name="arin")
            ar_out = dpool.tile([128, 260], f32, addr_space="Shared",
                                tag="arout", name="arout")

            identsb = cpool.tile([128, 128], f32)
            nc.sync.dma_start(out=identsb[:], in_=ident_t[:, :])
            identbf = cpool.tile([128, 128], bf)
            nc.vector.tensor_copy(out=identbf[:], in_=identsb[:])
            onesb = cpool.tile([128, 1], f32)
            nc.sync.dma_start(out=onesb[:], in_=ones_t[:, :])
            gmsb = cpool.tile([128, 1], f32)
            nc.sync.dma_start(out=gmsb[:], in_=gmask_t[:, :])
            mwsb = cpool.tile([128, c.NB], f32)
            nc.sync.dma_start(out=mwsb[:], in_=mw_t[:, :])
            w1sb = [[cpool.tile([128, 512], bf, tag=f"w1_{d}{v}",
                                name=f"w1_{d}{v}") for v in range(2)]
                    for d in range(2)]
            w2sb = [[cpool.tile([128, 64], bf, tag=f"w2_{d}{v}",
                                name=f"w2_{d}{v}") for v in range(2)]
                    for d in range(2)]
            b1csb = [cpool.tile([128, 2], f32, tag=f"b1c_{d}", name=f"b1c_{d}")
                     for d in range(2)]
            b2sb = [cpool.tile([128, 128], f32, tag=f"b2_{d}", name=f"b2_{d}")
                    for d in range(2)]
            for d in range(2):
                nc.sync.dma_start(out=w1sb[d][0][:], in_=w1on_t[d][:, :])
                nc.sync.dma_start(out=w1sb[d][1][:], in_=w1tg_t[d][:, :])
                nc.sync.dma_start(out=w2sb[d][0][:], in_=w2on_t[d][:, :])
                nc.sync.dma_start(out=w2sb[d][1][:], in_=w2tg_t[d][:, :])
                nc.sync.dma_start(out=b1csb[d][:], in_=b1c_t[d][:, :])
                nc.sync.dma_start(out=b2sb[d][:], in_=b2bc_t[d][:, :])

            stash = [cpool.tile([128, c.NB * 128], bf, tag=f"st{d}",
                                name=f"st{d}") for d in range(2)]
            cdot = cpool.tile([128, c.NB], f32)
            cd2 = cpool.tile([128, c.NB], f32)
            cn1 = cpool.tile([128, c.NB], f32)
            cn1b = cpool.tile([128, c.NB], f32)
            cn2 = cpool.tile([128, c.NB], f32)
            cn2b = cpool.tile([128, c.NB], f32)

            # ================= L1 (per dir) ===========================
            def l1_dir(d, pools):
                xep, s1p, fzp, pap, php, ptp = pools
                kt1 = KT1[d]
                kt1s = meta["KT1S"][d]
                secoff = meta["SECOFF"][d]
                NSEC = meta["NSEC"]
                ord1 = meta["ord1"][d]
                kt1off = np.r_[0, np.cumsum(kt1)].astype(np.int64)
                if True:
                    for g in range(c.NG):
                        b0 = g * c.GB1
                        blks = list(range(b0, min(b0 + c.GB1, c.NB)))
                        k0, k1 = int(kt1off[b0]), int(kt1off[blks[-1] + 1])
                        nkt = k1 - k0
                        xet = xep.tile([128, nkt * 512], bf, tag="xe",
                                       name="xe")
                        nc.sync.dma_start(
                            out=xet[:], in_=xe_t[d][:, k0 * 512:k1 * 512])
                        s1s = s1p.tile([128, nkt * 128], bf, tag="s1",
                                       name="s1")
                        nc.sync.dma_start(
                            out=s1s[:], in_=s1_t[d][:, k0 * 128:k1 * 128])
                        gw = len(blks) * 128
                        sbA = [fzp.tile([128, 4 * c.GB1 * 128], bf,
                                        tag=f"sbA{v}", name=f"sbA{v}")
                               for v in range(2)]
                        for bi, b in enumerate(blks):
                            nsec = kt1s[b]          # [NSEC] tile counts
                            psA = [pap.tile([128, 512], f32, tag=f"pA{s}",
                                            name=f"pA{s}")
                                   for s in range(NSEC)]
                            for s in range(NSEC):
                                for t in range(int(nsec[s])):
                                    kt = (int(kt1off[b]) + int(secoff[b, s])
                                          + t - k0)
                                    nc.tensor.matmul(
                                        out=psA[s][:],
                                        lhsT=s1s[:, kt * 128:(kt + 1) * 128],
                                        rhs=xet[:, kt * 512:(kt + 1) * 512],
                                        start=(t == 0),
                                        stop=(t == int(nsec[s]) - 1))
                            # variant A's: on = um (+tok), tgt = um + mk
                            asb = [fzp.tile([128, 512], bf, tag=f"as{v}",
                                            name=f"as{v}") for v in range(2)]
                            ntok = int(nsec[2]) if NSEC > 2 else 0
                            # on-variant base: um (DVE reads at most one
                            # PSUM input, so stage through SBUF)
                            if nsec[0] > 0:
                                nc.scalar.copy(out=asb[0][:], in_=psA[0][:])
                            elif ntok > 0:
                                nc.scalar.copy(out=asb[0][:], in_=psA[2][:])
                            else:
                                nc.vector.memset(asb[0][:], 0.0)
                            # tgt-variant: um + mk (um from SBUF copy)
                            if nsec[0] > 0 and nsec[1] > 0:
                                nc.vector.tensor_tensor(
                                    out=asb[1][:], in0=asb[0][:],
                                    in1=psA[1][:], op=ADD)
                            elif nsec[1] > 0:
                                nc.scalar.copy(out=asb[1][:], in_=psA[1][:])
                            elif nsec[0] > 0:
                                nc.vector.tensor_copy(out=asb[1][:],
                                                      in_=asb[0][:])
                            else:
                                nc.vector.memset(asb[1][:], 0.0)
                            # token correction onto the on-variant (after
                            # the tgt add consumed the plain um copy)
                            if nsec[0] > 0 and ntok > 0:
                                nc.vector.tensor_tensor(
                                    out=asb[0][:], in0=asb[0][:],
                                    in1=psA[2][:], op=ADD)
                            for v in range(2):
                                trt = ptp.tile([128, 512], bf, tag="tr",
                                               name="tr")
                                for ch in range(4):
                                    nc.tensor.transpose(
                                        out=trt[:, ch * 128:(ch + 1) * 128],
                                        in_=asb[v][:, ch * 128:(ch + 1) * 128],
                                        identity=identbf[:])
                                nc.vector.tensor_copy(
                                    out=sbA[v][:].rearrange(
                                        "p (ch n) -> p ch n",
                                        ch=4)[:, :, bi * 128:(bi + 1) * 128],
                                    in_=trt[:].rearrange(
                                        "p (ch n) -> p ch n", ch=4))
                        hT = [php.tile([128, c.GB1 * 128], f32, tag=f"hT{v}",
                                       name=f"hT{v}") for v in range(2)]
                        for v in range(2):
                            for ch in range(4):
                                nc.tensor.matmul(
                                    out=hT[v][:, 0:gw],
                                    lhsT=w1sb[d][v][:, ch * 128:(ch + 1) * 128],
                                    rhs=sbA[v][:].rearrange(
                                        "p (ch n) -> p ch n",
                                        ch=4)[:, ch, 0:gw],
                                    start=(ch == 0), stop=(ch == 3))
                        hsb = [fzp.tile([128, c.GB1 * 128], bf, tag=f"h{v}",
                                        name=f"h{v}") for v in range(2)]
                        for v in range(2):
                            nc.scalar.activation(
                                out=hsb[v][:, 0:gw], in_=hT[v][:, 0:gw],
                                func=AF.Relu, bias=b1csb[d][:, v:v + 1])
                        z2T = php.tile([128, c.GB1 * 128], f32, tag="z2T",
                                       name="z2T")
                        for v in range(2):
                            nc.tensor.matmul(
                                out=z2T[v * 64:(v + 1) * 64, 0:gw],
                                lhsT=w2sb[d][v][:], rhs=hsb[v][:, 0:gw],
                                start=True, stop=True)
                        z2Tsb = fzp.tile([128, c.GB1 * 128], bf, tag="z2Tsb",
                                         name="z2Tsb")
                        nc.vector.tensor_copy(out=z2Tsb[:, 0:gw],
                                              in_=z2T[:, 0:gw])
                        trz = ptp.tile([128, c.GB1 * 128], bf, tag="tr",
                                       name="trz")
                        for bi, b in enumerate(blks):
                            nc.tensor.transpose(
                                out=trz[:, bi * 128:(bi + 1) * 128],
                                in_=z2Tsb[:, bi * 128:(bi + 1) * 128],
                                identity=identbf[:])
                        for bi, b in enumerate(blks):
                            ph = int(ord1[b])
                            nc.vector.tensor_copy(
                                out=stash[d][:, ph * 128:(ph + 1) * 128],
                                in_=trz[:, bi * 128:(bi + 1) * 128])
                            nc.sync.dma_start(
                                out=z2own[d][ph * 128:(ph + 1) * 128, :],
                                in_=stash[d][:, ph * 128:(ph + 1) * 128])

            with (
                tc.tile_pool(name="xe", bufs=2) as xep,
                tc.tile_pool(name="s1p", bufs=2) as s1p,
                tc.tile_pool(name="fz", bufs=2) as fzp,
                tc.tile_pool(name="pA", bufs=2, space="PSUM") as pap,
                tc.tile_pool(name="pH", bufs=1, space="PSUM") as php,
                tc.tile_pool(name="pT", bufs=1, space="PSUM") as ptp,
            ):
                pools1 = (xep, s1p, fzp, pap, php, ptp)
                l1_dir(0, pools1)
                nc.gpsimd.collective_compute(
                    "AllGather", mybir.AluOpType.bypass,
                    replica_groups=[list(range(c.C))],
                    ins=[z2own[0].opt()], outs=[z2full[0].opt()])
                l1_dir(1, pools1)
                nc.gpsimd.collective_compute(
                    "AllGather", mybir.AluOpType.bypass,
                    replica_groups=[list(range(c.C))],
                    ins=[z2own[1].opt()], outs=[z2full[1].opt()])

            # ================= L2 (per dir) ===========================
            with tc.tile_pool(name="plps", bufs=1, space="PSUM") as plp:
                pool_ps = [plp.tile([128, 128], f32, tag=f"pl{d}",
                                    name=f"pl{d}") for d in range(2)]

                def l2_dir(d, pools):
                    gzp, ixp, s2p, sdp, fhp, pbp = pools
                    wlen = lambda w: min(c.WIN, c.NPAD - w * c.WIN)
                    pool_mm = [0]
                    tot_pool = c.NB
                    if True:
                        for sg, ops in enumerate(sched2[d]):
                            bm = bandmeta[d]["per_sg"][sg]
                            blocks, bops, nmm = (bm["blocks"], bm["ops"],
                                                 bm["nmm"])
                            if ops:
                                sk0 = ops[0][1]
                                sk1 = ops[-1][1] + ops[-1][2]
                            else:
                                sk0 = sk1 = 0
                            snk = max(sk1 - sk0, 1)
                            if bops:
                                bd0 = bops[0][1]
                                bd1 = bops[-1][1] + 1
                            else:
                                bd0 = bd1 = 0
                            nbd = max(bd1 - bd0, 1)
                            gzt = gzp.tile([128, snk * 128], bf, tag="gz",
                                           name="gz")
                            it = ixp.tile([128, snk * 8], i16, tag="ix",
                                          name="ix")
                            s2s = s2p.tile([128, nbd * 128], bf, tag="s2",
                                           name="s2")
                            sds = sdp.tile([128, len(blocks) * 128], bf,
                                           tag="sd", name="sd")
                            b00 = blocks[0]
                            nc.sync.dma_start(
                                out=sds[:],
                                in_=sd_t[d][:, b00 * 128:
                                            (b00 + len(blocks)) * 128])
                            pps = sdp.tile([128, len(blocks) * 128], bf,
                                           tag="pp", name="pp")
                            nc.sync.dma_start(
                                out=pps[:],
                                in_=pp_t[:, b00 * 128:
                                         (b00 + len(blocks)) * 128])
                            if sk1 > sk0:
                                nc.sync.dma_start(
                                    out=it[:, 0:(sk1 - sk0) * 8],
                                    in_=i2_t[d][:, sk0 * 8:sk1 * 8])
                            if bd1 > bd0:
                                nc.sync.dma_start(
                                    out=s2s[:, 0:(bd1 - bd0) * 128],
                                    in_=s2_t[d][:, bd0 * 128:bd1 * 128])
                            for w, base, nk in ops:
                                o = base - sk0
                                nc.gpsimd.dma_gather(
                                    gzt[:, o * 128:(o + nk) * 128].rearrange(
                                        "p (k e) -> p k e", k=nk, e=128),
                                    z2full[d][w * c.WIN:w * c.WIN + wlen(w), :],
                                    it[:, o * 8:(o + nk) * 8],
                                    nk * 128, nk * 128, 128,
                                    elem_step=None, single_packet=True)
                            # per-block psum slices of 2 group tiles:
                            # self first, then bands
                            psG = [pbp.tile([128, c.GB1 * 128], f32,
                                            tag=f"psG{j}", name=f"psG{j}")
                                   for j in range(c.SGF)]

                            def bslice(b):
                                gi = (b // c.GB1) % c.SGF
                                bi = b % c.GB1
                                return psG[gi][:, bi * 128:(bi + 1) * 128]

                            done = {}
                            for bi, b in enumerate(blocks):
                                nc.tensor.matmul(
                                    out=bslice(b),
                                    lhsT=sds[:, bi * 128:(bi + 1) * 128],
                                    rhs=stash[d][:, b * 128:(b + 1) * 128],
                                    start=True, stop=(nmm[b] == 1),
                                    skip_group_check=True)
                                done[b] = 1
                            for (ko, band, b) in bops:
                                nc.tensor.matmul(
                                    out=bslice(b),
                                    lhsT=s2s[:, (band - bd0) * 128:
                                             (band - bd0 + 1) * 128],
                                    rhs=gzt[:, (ko - sk0) * 128:
                                            (ko - sk0 + 1) * 128],
                                    start=False,
                                    stop=(done[b] + 1 == nmm[b]),
                                    skip_group_check=True)
                                done[b] += 1
                            # finalize per block
                            for bi, b in enumerate(blocks):
                                h2b = fhp.tile([128, 128], bf, tag="h2b",
                                               name="h2b")
                                nc.vector.tensor_tensor(
                                    out=h2b[:], in0=bslice(b),
                                    in1=b2sb[d][:], op=ADD)
                                nc.tensor.matmul(
                                    out=pool_ps[d][:],
                                    lhsT=pps[:, bi * 128:(bi + 1) * 128],
                                    rhs=h2b[:],
                                    start=(pool_mm[0] == 0),
                                    stop=(pool_mm[0] == tot_pool - 1),
                                    skip_group_check=True)
                                pool_mm[0] += 1
                                if d == 0:
                                    nc.vector.tensor_copy(
                                        out=stash[0][:, b * 128:(b + 1) * 128],
                                        in_=h2b[:])
                                else:
                                    td = stash[0][:, b * 128:(b + 1) * 128]
                                    sc = fhp.tile([128, 64], f32, tag="sc",
                                                  name="sc")
                                    sc2 = fhp.tile([128, 64], f32, tag="sc2",
                                                   name="sc2")
                                    nc.vector.scalar_tensor_tensor(
                                        out=sc[:], in0=td[:, 0:64], scalar=1.0,
                                        in1=td[:, 64:128], op0=MUL, op1=MUL,
                                        accum_out=cdot[:, b:b + 1])
                                    nc.vector.scalar_tensor_tensor(
                                        out=sc2[:], in0=h2b[:, 0:64],
                                        scalar=1.0, in1=h2b[:, 64:128],
                                        op0=MUL, op1=MUL,
                                        accum_out=cd2[:, b:b + 1])
                                    nc.vector.scalar_tensor_tensor(
                                        out=sc[:], in0=td[:, 0:64], scalar=1.0,
                                        in1=td[:, 0:64], op0=MUL, op1=MUL,
                                        accum_out=cn1[:, b:b + 1])
                                    nc.vector.scalar_tensor_tensor(
                                        out=sc2[:], in0=h2b[:, 0:64],
                                        scalar=1.0, in1=h2b[:, 0:64],
                                        op0=MUL, op1=MUL,
                                        accum_out=cn1b[:, b:b + 1])
                                    nc.vector.scalar_tensor_tensor(
                                        out=sc[:], in0=td[:, 64:128],
                                        scalar=1.0, in1=td[:, 64:128],
                                        op0=MUL, op1=MUL,
                                        accum_out=cn2[:, b:b + 1])
                                    nc.vector.scalar_tensor_tensor(
                                        out=sc2[:], in0=h2b[:, 64:128],
                                        scalar=1.0, in1=h2b[:, 64:128],
                                        op0=MUL, op1=MUL,
                                        accum_out=cn2b[:, b:b + 1])

                with (
                    tc.tile_pool(name="gz", bufs=2) as gzp,
                    tc.tile_pool(name="ix", bufs=2) as ixp,
                    tc.tile_pool(name="s2p", bufs=2) as s2p,
                    tc.tile_pool(name="sdp", bufs=2) as sdp,
                    tc.tile_pool(name="fh", bufs=3) as fhp,
                    tc.tile_pool(name="pB", bufs=2, space="PSUM") as pbp,
                ):
                    pools2 = (gzp, ixp, s2p, sdp, fhp, pbp)
                    l2_dir(0, pools2)
                    l2_dir(1, pools2)

                # ============ masked cosine tail + pool AR ============
                with (
                    tc.tile_pool(name="tail", bufs=2) as tlp,
                    tc.tile_pool(name="tps", bufs=2, space="PSUM") as tpp,
                ):
                    dot = tlp.tile([128, c.NB], f32, tag="dot", name="dot")
                    nc.vector.tensor_tensor(out=dot[:], in0=cdot[:],
                                            in1=cd2[:], op=ADD)
                    n1 = tlp.tile([128, c.NB], f32, tag="n1", name="n1")
                    nc.vector.tensor_tensor(out=n1[:], in0=cn1[:], in1=cn1b[:],
                                            op=ADD)
                    n2 = tlp.tile([128, c.NB], f32, tag="n2", name="n2")
                    nc.vector.tensor_tensor(out=n2[:], in0=cn2[:], in1=cn2b[:],
                                            op=ADD)

                    def rcp_sqrt(nt, tag):
                        r = tlp.tile([128, c.NB], f32, tag=tag, name=tag)
                        nc.scalar.sqrt(out=r[:], in_=nt[:])
                        nc.vector.tensor_scalar_max(out=r[:], in0=r[:],
                                                    scalar1=1e-12)
                        nc.vector.reciprocal(out=r[:], in_=r[:])
                        return r

                    r1 = rcp_sqrt(n1, "r1")
                    r2 = rcp_sqrt(n2, "r2")
                    cosv = tlp.tile([128, c.NB], f32, tag="cosv", name="cosv")
                    nc.vector.tensor_tensor(out=cosv[:], in0=dot[:], in1=r1[:],
                                            op=MUL)
                    nc.vector.tensor_tensor(out=cosv[:], in0=cosv[:],
                                            in1=r2[:], op=MUL)
                    term = tlp.tile([128, c.NB], f32, tag="term", name="term")
                    macc = tlp.tile([128, 1], f32, tag="macc", name="macc")
                    nc.vector.tensor_tensor(out=term[:], in0=mwsb[:],
                                            in1=cosv[:], op=MUL)
                    nc.vector.scalar_tensor_tensor(
                        out=term[:], in0=mwsb[:], scalar=1.0, in1=term[:],
                        op0=MUL, op1=SUB, accum_out=macc[:])
                    msps = tpp.tile([1, 1], f32, tag="ms", name="ms")
                    nc.tensor.matmul(out=msps[:], lhsT=macc[:], rhs=onesb[:],
                                     start=True, stop=True)

                    arsb = tlp.tile([128, 260], f32, tag="arsb", name="arsb")
                    nc.vector.memset(arsb[:], 0.0)
                    nc.vector.tensor_copy(out=arsb[:, 0:128],
                                          in_=pool_ps[0][:])
                    nc.vector.tensor_copy(out=arsb[:, 128:256],
                                          in_=pool_ps[1][:])
                    nc.vector.tensor_copy(out=arsb[0:1, 256:257], in_=msps[:])
                    nc.sync.dma_start(out=ar_in[:, :], in_=arsb[:])
                    nc.gpsimd.collective_compute(
                        "AllReduce", mybir.AluOpType.add,
                        replica_groups=[list(range(c.C))],
                        ins=[ar_in.opt()], outs=[ar_out.opt()])
                    ar2 = tlp.tile([128, 260], f32, tag="ar2", name="ar2")
                    nc.sync.dma_start(out=ar2[:], in_=ar_out[:, :])

                    def gacc(u0, u1, v0, v1, tag):
                        a1 = tlp.tile([128, 1], f32, tag=f"{tag}a",
                                      name=f"{tag}a")
                        a2 = tlp.tile([128, 1], f32, tag=f"{tag}b",
                                      name=f"{tag}b")
                        scr = tlp.tile([128, 64], f32, tag=f"{tag}s",
                                       name=f"{tag}s")
                        nc.vector.scalar_tensor_tensor(
                            out=scr[:], in0=u0, scalar=1.0, in1=v0,
                            op0=MUL, op1=MUL, accum_out=a1[:])
                        nc.vector.scalar_tensor_tensor(
                            out=scr[:], in0=u1, scalar=1.0, in1=v1,
                            op0=MUL, op1=MUL, accum_out=a2[:])
                        s = tlp.tile([128, 1], f32, tag=f"{tag}c",
                                     name=f"{tag}c")
                        nc.vector.tensor_tensor(out=s[:], in0=a1[:],
                                                in1=a2[:], op=ADD)
                        return s

                    tdon, tdtg = ar2[:, 0:64], ar2[:, 64:128]
                    buon, butg = ar2[:, 128:192], ar2[:, 192:256]
                    gdot = gacc(tdon, buon, tdtg, butg, "gd")
                    gn1 = gacc(tdon, buon, tdon, buon, "g1")
                    gn2 = gacc(tdtg, butg, tdtg, butg, "g2")

                    def rcp1(nt, tag):
                        r = tlp.tile([128, 1], f32, tag=tag, name=tag)
                        nc.scalar.sqrt(out=r[:], in_=nt[:])
                        nc.vector.tensor_scalar_max(out=r[:], in0=r[:],
                                                    scalar1=1e-12)
                        nc.vector.reciprocal(out=r[:], in_=r[:])
                        return r

                    gr1 = rcp1(gn1, "gr1")
                    gr2 = rcp1(gn2, "gr2")
                    cosg = tlp.tile([128, 1], f32, tag="cosg", name="cosg")
                    nc.vector.tensor_tensor(out=cosg[:], in0=gdot[:],
                                            in1=gr1[:], op=MUL)
                    nc.vector.tensor_tensor(out=cosg[:], in0=cosg[:],
                                            in1=gr2[:], op=MUL)
                    gterm = tlp.tile([128, 1], f32, tag="gt", name="gt")
                    nc.vector.tensor_scalar(out=gterm[:], in0=cosg[:],
                                            scalar1=-1.0, scalar2=1.0,
                                            op0=MUL, op1=ADD)
                    nc.vector.tensor_tensor(out=gterm[:], in0=gterm[:],
                                            in1=gmsb[:], op=MUL)
                    gsps = tpp.tile([1, 1], f32, tag="gs", name="gs")
                    nc.tensor.matmul(out=gsps[:], lhsT=gterm[:], rhs=onesb[:],
                                     start=True, stop=True)
                    l1t = tlp.tile([1, 1], f32, tag="l1", name="l1")
                    nc.scalar.activation(out=l1t[:], in_=gsps[:], func=AF.Copy,
                                         scale=1.0 / c.G)
                    l2t = tlp.tile([1, 1], f32, tag="l2", name="l2")
                    nc.scalar.activation(out=l2t[:], in_=ar2[0:1, 256:257],
                                         func=AF.Copy, scale=1.0 / c.M)
                    nc.vector.tensor_tensor(out=l1t[:], in0=l1t[:],
                                            in1=l2t[:], op=ADD)
                    nc.sync.dma_start(out=loss_t[:, :], in_=l1t[:])

    return nc


# ---------------------------------------------------------------- entry

LAST_RESULT = None


def kernel(_trace=False, **inputs):
    global LAST_RESULT
    import time
    from concourse import bass_utils
    cfg = FULL
    t0 = time.monotonic()
    meta, in_maps = host_prep(cfg, inputs)
    t1 = time.monotonic()
    nc = build_program(cfg, meta)
    t2 = time.monotonic()
    nc.compile()
    t3 = time.monotonic()
    res = bass_utils.run_bass_kernel_spmd(
        nc, in_maps, core_ids=list(range(cfg.C)),
        trace=_trace, trace_cores=[0] if _trace else None)
    t4 = time.monotonic()
    print(f"[kernel] prep {t1-t0:.1f}s build {t2-t1:.1f}s "
          f"compile {t3-t2:.1f}s run {t4-t3:.1f}s", file=sys.stderr)
    LAST_RESULT = res
    return np.float32(res.results[0]["loss"][0, 0])
